# revision 5
# baseline (speedup 1.0000x reference)
"""DFlashAttention Trainium2 kernel: 8-way tensor-parallel over heads.

Per core c (4 heads): Q-proj (f32r), fused double-RMSNorm, RoPE, full
attention of 64 draft queries over 4160 keys (bf16 QK/PV with the score
factorization  score = sum_e k[t,e]*(q_rope[e]*cos[t,e] + qtil[e]*ssw[t,e]),
qtil = -rot(q_rope), ssw = half-swapped sin), softmax w/o max-subtraction
(scores are O(1)), denominator via a ones-column fused into PV, and the
row-sharded Wo projection (f32r). Host sums the 8 partial outputs.
"""

import numpy as np
import ml_dtypes

import concourse.bacc as bacc
import concourse.tile as tile
import concourse.mybir as mybir
from concourse.bass_utils import run_bass_kernel_spmd

bf16 = ml_dtypes.bfloat16
F32 = mybir.dt.float32
F32R = mybir.dt.float32r
BF = mybir.dt.bfloat16
AF = mybir.ActivationFunctionType
MUL = mybir.AluOpType.mult
AXX = mybir.AxisListType.X

B, QL, CTX, HID, H, D = 4, 64, 4096, 4096, 32, 128
T = CTX + QL            # 4160
R = B * QL              # 256
HL = 4                  # heads per core
NC = 8                  # cores
EPS = 1e-6
SCALE = 1.0 / float(np.sqrt(D))
NT_FULL = T // 128      # 32 full t-tiles
TW_LAST = T - NT_FULL * 128  # 64


def build_program():
    nc = bacc.Bacc("TRN2", target_bir_lowering=False, debug=False)

    kt = nc.dram_tensor("kt", [B * HL, 128, T], F32, kind="ExternalInput")
    hst = nc.dram_tensor("hst", [HID, R], F32R, kind="ExternalInput")
    wq = nc.dram_tensor("wq", [HID, HL * D], F32R, kind="ExternalInput")
    wo = nc.dram_tensor("wo", [HL * D, HID], F32R, kind="ExternalInput")
    cost = nc.dram_tensor("cost", [128, T], F32, kind="ExternalInput")
    sinsw = nc.dram_tensor("sinsw", [128, T], F32, kind="ExternalInput")
    cqn = nc.dram_tensor("cqn", [128, 128], F32, kind="ExternalInput")
    sqn = nc.dram_tensor("sqn", [128, 128], F32, kind="ExternalInput")
    eyef = nc.dram_tensor("eyef", [128, 128], F32, kind="ExternalInput")
    eyeb = nc.dram_tensor("eyeb", [128, 128], BF, kind="ExternalInput")
    outp = nc.dram_tensor("outp", [R, HID], F32, kind="ExternalOutput")

    with tile.TileContext(nc) as tc:
        with (
            tc.tile_pool(name="singles", bufs=1) as sg,
            tc.tile_pool(name="ktp", bufs=2) as ktp,
            tc.tile_pool(name="kcp", bufs=2) as kcp,
            tc.tile_pool(name="ksp", bufs=2) as ksp,
        ):
            # ---- long-lived small tensors ----
            cost_bf = sg.tile([128, T], BF)
            nc.gpsimd.dma_start(out=cost_bf, in_=cost.ap())
            sinsw_bf = sg.tile([128, T], BF)
            nc.gpsimd.dma_start(out=sinsw_bf, in_=sinsw.ap())
            cqn_sb = sg.tile([128, 128], F32)
            nc.sync.dma_start(out=cqn_sb, in_=cqn.ap())
            sqn_sb = sg.tile([128, 128], F32)
            nc.sync.dma_start(out=sqn_sb, in_=sqn.ap())
            eyef_sb = sg.tile([128, 128], F32)
            nc.sync.dma_start(out=eyef_sb, in_=eyef.ap())
            eyeb_sb = sg.tile([128, 128], BF)
            nc.sync.dma_start(out=eyeb_sb, in_=eyeb.ap())
            oT_all = sg.tile([128, HL, B, QL], F32R, tag="oT")
            eps2_sb = sg.tile([128, 1], F32, tag="eps2")
            nc.vector.memset(eps2_sb, EPS * EPS)
            qc_sb = [sg.tile([128, R], BF, tag=f"qc{h}", name=f"qc{h}") for h in range(HL)]
            qs_sb = [sg.tile([128, R], BF, tag=f"qs{h}", name=f"qs{h}") for h in range(HL)]
            # big weight slot: Wq first, then Wo reuses it
            wq_sb = sg.tile([128, 32, HL * D], F32R, tag="bigw")

            # =========== Phase B: Q path ===========
            with (
                tc.tile_pool(name="bsb", bufs=2) as bsb,
                tc.tile_pool(name="hstp", bufs=1) as hstp,
                tc.tile_pool(name="psB", bufs=2, space="PSUM") as psB,
            ):
                hst_sb = hstp.tile([128, 32, R], F32R)
                nc.sync.dma_start(
                    out=hst_sb, in_=hst.ap().rearrange("(k p) r -> p k r", p=128)
                )
                nc.sync.dma_start(
                    out=wq_sb, in_=wq.ap().rearrange("(k p) m -> p k m", p=128)
                )

                for h in range(HL):
                    psq = psB.tile([128, R], F32, tag="psq")
                    for k in range(32):
                        nc.tensor.matmul(
                            psq,
                            wq_sb[:, k, h * D:(h + 1) * D],
                            hst_sb[:, k, :],
                            start=(k == 0),
                            stop=(k == 31),
                        )
                    qt_sb = bsb.tile([128, R], F32, tag="qt")
                    nc.scalar.copy(out=qt_sb, in_=psq)
                    pnat = psB.tile([128, 2, 128], F32, tag="pnat")
                    nc.tensor.transpose(pnat[:, 0, :], qt_sb[:, 0:128], eyef_sb)
                    nc.tensor.transpose(pnat[:, 1, :], qt_sb[:, 128:256], eyef_sb)
                    qsq = bsb.tile([128, 2, 128], F32, tag="qsq")
                    nc.scalar.activation(out=qsq, in_=pnat, func=AF.Square)
                    ssq = bsb.tile([128, 2], F32, tag="ssq")
                    nc.vector.reduce_sum(ssq, qsq, axis=AXX)
                    fcol = bsb.tile([128, 2], F32, tag="fcol")
                    # double-RMSNorm(w=1) == x * rsqrt((1+eps)*m + eps^2), m = ssq/128
                    nc.scalar.activation(
                        out=fcol, in_=ssq, func=AF.Sqrt,
                        scale=(1.0 + EPS) / D, bias=eps2_sb[:, 0:1],
                    )
                    nc.vector.reciprocal(fcol, fcol)
                    qn = bsb.tile([128, 2, 128], F32, tag="qn")
                    nc.vector.tensor_scalar_mul(qn[:, 0, :], pnat[:, 0, :], fcol[:, 0:1])
                    nc.vector.tensor_scalar_mul(qn[:, 1, :], pnat[:, 1, :], fcol[:, 1:2])
                    # rope in natural layout (d on free axis)
                    qcn = bsb.tile([128, 2, 128], BF, tag="qcn")
                    qsn = bsb.tile([128, 2, 128], BF, tag="qsn")
                    for j in range(2):
                        a_t = bsb.tile([128, 128], F32, tag="ropeA")
                        nc.vector.tensor_mul(a_t, qn[:, j, :], cqn_sb)
                        b_t = bsb.tile([128, 128], F32, tag="ropeB")
                        nc.vector.tensor_mul(b_t[:, 0:64], qn[:, j, 64:128], sqn_sb[:, 0:64])
                        nc.vector.tensor_mul(b_t[:, 64:128], qn[:, j, 0:64], sqn_sb[:, 64:128])
                        # q_rope = a -/+ b  (sign of rotate_half folded here)
                        nc.vector.tensor_sub(qcn[:, j, 0:64], a_t[:, 0:64], b_t[:, 0:64])
                        nc.vector.tensor_add(qcn[:, j, 64:128], a_t[:, 64:128], b_t[:, 64:128])
                        # qtil = -rot(q_rope)
                        nc.vector.tensor_copy(qsn[:, j, 0:64], qcn[:, j, 64:128])
                        nc.vector.tensor_scalar_mul(qsn[:, j, 64:128], qcn[:, j, 0:64], -1.0)
                    pqc = psB.tile([128, R], BF, tag="pqc")
                    pqs = psB.tile([128, R], BF, tag="pqs")
                    for j in range(2):
                        nc.tensor.transpose(pqc[:, j * 128:(j + 1) * 128], qcn[:, j, :], eyeb_sb)
                        nc.tensor.transpose(pqs[:, j * 128:(j + 1) * 128], qsn[:, j, :], eyeb_sb)
                    nc.scalar.copy(out=qc_sb[h], in_=pqc)
                    nc.scalar.copy(out=qs_sb[h], in_=pqs)

            # =========== Phase C: attention over K ===========
            with (
                tc.tile_pool(name="expp", bufs=3) as expp,
                tc.tile_pool(name="vp", bufs=3) as vp,
                tc.tile_pool(name="osm", bufs=2) as osm,
                tc.tile_pool(name="psSC", bufs=2, space="PSUM") as psSC,
                tc.tile_pool(name="psV", bufs=2, space="PSUM") as psV,
                tc.tile_pool(name="psO", bufs=2, space="PSUM") as psO,
                tc.tile_pool(name="psOT", bufs=2, space="PSUM") as psOT,
            ):
                wo_sb = sg.tile([128, HL, HID], F32R, tag="bigw")
                nc.sync.dma_start(
                    out=wo_sb, in_=wo.ap().rearrange("(k p) m -> p k m", p=128)
                )
                for i in range(B * HL):
                    b, h = divmod(i, HL)
                    kt_bf = ktp.tile([128, T], BF, tag="kt")
                    nc.gpsimd.dma_start(out=kt_bf, in_=kt.ap()[i])
                    kc = kcp.tile([128, T], BF, tag="kc")
                    nc.vector.tensor_mul(kc, kt_bf, cost_bf)
                    ks = ksp.tile([128, T], BF, tag="ks")
                    nc.vector.tensor_mul(ks, kt_bf, sinsw_bf)

                    po = psO.tile([64, 129], F32, tag="po")
                    qcb = qc_sb[h][:, b * QL:(b + 1) * QL]
                    qsb = qs_sb[h][:, b * QL:(b + 1) * QL]

                    ntiles = NT_FULL + 1
                    for g in range((ntiles + 7) // 8):  # groups of 8 t-tiles
                        j0, j1 = g * 8, min(ntiles, g * 8 + 8)
                        gw = j1 - j0
                        psc = psSC.tile([128, 8, QL], F32, tag="psc")
                        for jj in range(gw):
                            j = j0 + jj
                            t0 = j * 128
                            tw = 128 if j < NT_FULL else TW_LAST
                            nc.tensor.matmul(
                                psc[:tw, jj, :], kc[:, t0:t0 + tw], qcb,
                                start=True, stop=False,
                            )
                            nc.tensor.matmul(
                                psc[:tw, jj, :], ks[:, t0:t0 + tw], qsb,
                                start=False, stop=True,
                            )
                        exp_sb = expp.tile([128, 8, QL], BF, tag="exp")
                        if j1 == ntiles:  # last group contains the 64-wide tile
                            if gw > 1:
                                nc.scalar.activation(
                                    out=exp_sb[:, 0:gw - 1, :], in_=psc[:, 0:gw - 1, :],
                                    func=AF.Exp, scale=SCALE,
                                )
                            nc.scalar.activation(
                                out=exp_sb[:TW_LAST, gw - 1, :], in_=psc[:TW_LAST, gw - 1, :],
                                func=AF.Exp, scale=SCALE,
                            )
                        else:
                            nc.scalar.activation(
                                out=exp_sb[:, 0:gw, :], in_=psc[:, 0:gw, :],
                                func=AF.Exp, scale=SCALE,
                            )
                        # V tiles: transpose kt back to [t, d], batches of 4
                        for vg in range((gw + 3) // 4):
                            vj0, vj1 = j0 + vg * 4, min(j1, j0 + vg * 4 + 4)
                            pv = psV.tile([128, 4, 128], BF, tag="pv")
                            for jj in range(vj1 - vj0):
                                j = vj0 + jj
                                t0 = j * 128
                                tw = 128 if j < NT_FULL else TW_LAST
                                nc.tensor.transpose(
                                    pv[:tw, jj, :], kt_bf[:, t0:t0 + tw], eyeb_sb
                                )
                            v_sb = vp.tile([128, 4, 130], BF, tag="v")
                            nvw = vj1 - vj0
                            nc.scalar.copy(
                                out=v_sb[:, 0:nvw, 0:128], in_=pv[:, 0:nvw, :]
                            )
                            nc.vector.memset(v_sb[:, 0:nvw, 128:129], 1.0)
                            for jj in range(nvw):
                                j = vj0 + jj
                                tw = 128 if j < NT_FULL else TW_LAST
                                nc.tensor.matmul(
                                    po,
                                    exp_sb[:tw, j - j0, :],
                                    v_sb[:tw, jj, 0:129],
                                    start=(j == 0),
                                    stop=(j == ntiles - 1),
                                )
                    # normalize + transpose out
                    rec = osm.tile([64, 1], F32, tag="rec")
                    nc.vector.reciprocal(rec, po[:, 128:129])
                    onrm = osm.tile([64, 128], F32, tag="onrm")
                    nc.vector.tensor_scalar_mul(onrm, po[:, 0:128], rec)
                    poT = psOT.tile([128, 64], F32, tag="poT")
                    nc.tensor.transpose(poT, onrm, eyef_sb[0:64, 0:64])
                    nc.scalar.copy(out=oT_all[:, h, b, :], in_=poT)

            # =========== Phase E: output projection ===========
            with (
                tc.tile_pool(name="obp", bufs=2) as obp,
                tc.tile_pool(name="psW", bufs=4, space="PSUM") as psW,
            ):
                for bp in range(2):  # batch pairs
                    ob = obp.tile([128, HID], F32, tag="ob")
                    for bo in range(2):
                        b = bp * 2 + bo
                        for oc in range(8):
                            pw = psW.tile([64, 512], F32, tag="pw")
                            for hd in range(HL):
                                nc.tensor.matmul(
                                    pw,
                                    oT_all[:, hd, b, :],
                                    wo_sb[:, hd, oc * 512:(oc + 1) * 512],
                                    start=(hd == 0),
                                    stop=(hd == HL - 1),
                                )
                            nc.vector.tensor_copy(
                                out=ob[bo * 64:(bo + 1) * 64, oc * 512:(oc + 1) * 512],
                                in_=pw,
                            )
                    nc.sync.dma_start(
                        out=outp.ap()[bp * 128:(bp + 1) * 128, :], in_=ob
                    )
    nc.compile()
    return nc


_PROGRAM = None


def get_program():
    global _PROGRAM
    if _PROGRAM is None:
        _PROGRAM = build_program()
    return _PROGRAM


def stage_inputs(hidden_states, target_hidden, cos, sin, Wqkv, Wo, q_norm_w):
    """Host-side shard + layout staging. Returns list of 8 in_maps."""
    assert np.allclose(np.asarray(q_norm_w), 1.0), "kernel assumes q_norm_w == ones"
    hs = np.asarray(hidden_states, np.float32)
    th = np.asarray(target_hidden, np.float32)
    cos = np.asarray(cos, np.float32)
    sin = np.asarray(sin, np.float32)
    Wqkv = np.asarray(Wqkv, np.float32)
    Wo = np.asarray(Wo, np.float32)

    kv = np.concatenate([th, hs], axis=1)                    # [B, T, HID]
    # [B, H, D, T], b-major per core slice
    kt_all = np.ascontiguousarray(kv.reshape(B, T, H, D).transpose(0, 2, 3, 1))
    hst = np.ascontiguousarray(hs.reshape(R, HID).T)          # [HID, R]
    cost = np.ascontiguousarray(cos.T)                        # [128, T]
    sinsw = np.ascontiguousarray(np.roll(sin.T, -64, axis=0))  # row e <- e+64 mod 128
    cqn = np.ascontiguousarray(np.tile(cos[CTX:], (2, 1)))    # [128, 128]
    sqn = np.ascontiguousarray(np.tile(sin[CTX:], (2, 1)))
    eyef = np.eye(128, dtype=np.float32)
    eyeb = np.eye(128, dtype=bf16)

    in_maps = []
    for c in range(NC):
        hsl = slice(c * HL, (c + 1) * HL)
        in_maps.append({
            "kt": np.ascontiguousarray(kt_all[:, hsl]).reshape(B * HL, D, T),
            "hst": hst,
            "wq": np.ascontiguousarray(Wqkv[:, c * HL * D:(c + 1) * HL * D]),
            "wo": np.ascontiguousarray(Wo[c * HL * D:(c + 1) * HL * D, :]),
            "cost": cost,
            "sinsw": sinsw,
            "cqn": cqn,
            "sqn": sqn,
            "eyef": eyef,
            "eyeb": eyeb,
        })
    return in_maps


def kernel(hidden_states, target_hidden, cos, sin, Wqkv, Wo, q_norm_w):
    nc = get_program()
    in_maps = stage_inputs(hidden_states, target_hidden, cos, sin, Wqkv, Wo, q_norm_w)
    res = run_bass_kernel_spmd(nc, in_maps, core_ids=list(range(NC)))
    out = np.zeros((R, HID), np.float32)
    for r in res.results:
        out += r["outp"]
    return out.reshape(B, QL, HID)
